# revision 8
# baseline (speedup 1.0000x reference)
"""DAEGC 2-layer GAT + inner-product decode on 8 Trainium2 NeuronCores.

Math (per reference):
  h1 = x @ W1;  e1 = (h1@a_s1) + (h1@a_n1)^T;  e1 = lrelu(e1*M); mask adj;
  attn1 = softmax(e1, axis=1); h1o = elu(attn1 @ h1)
  (same again with W2 -> h2o);  z = l2normalize(h2o); A = sigmoid(z z^T)

Sharding: row-shard N across 8 cores. Scores are computed *transposed*
(tile [128 j, NLOC i]) so the PV matmul needs no on-device transposes; the
host pre-transposes each core's adj/M column-block into one fp16 tensor.
Softmax row-sums ride the PV matmul as an appended ones-column. exp() is
applied without a running max (scores are bounded ~|13|; fp32-safe).

Key identity used: softmax numerator p = exp(lrelu((fs_i+fn_j)*M)) * adj,
with fs = x@(W1@a_self), fn = x@(W1@a_neighs) appended as extra columns of
W1 so one matmul produces h, fs, fn together.
"""

import os
import sys

import numpy as np

for _p in ("/opt/trn_rl_repo",):
    if _p not in sys.path:
        sys.path.insert(0, _p)

P = 128
ALPHA = 0.2
NCORES = 8

_compiled_cache = {}


def _build_program(N, F_IN, HID, EMB):
    from concourse import bacc, bass, masks, mybir, tile

    f32 = mybir.dt.float32
    f16 = mybir.dt.float16
    bf16 = mybir.dt.bfloat16
    ALU = mybir.AluOpType
    AF = mybir.ActivationFunctionType

    NT = N // P              # number of 128-row tiles over the full N
    NLOC = N // NCORES       # rows owned per core
    TLOC = NLOC // P         # own 128-row tiles per core
    KH1 = F_IN // P          # k-chunks for layer-1 feature matmul
    KH2 = HID // P           # k-chunks for layer-2 feature matmul
    C1 = HID + 2             # h1' columns: h | fs | fn
    C2 = EMB + 2
    R1 = HID + 1             # PV rhs cols for layer 1 (h | ones)
    R2 = EMB + 1

    assert NLOC % P == 0 and F_IN % P == 0 and HID % P == 0

    nc = bacc.Bacc(
        "TRN2",
        target_bir_lowering=False,
        debug=False,
        enable_asserts=False,
        num_devices=NCORES,
    )

    # ---- I/O ----
    xt_d = nc.dram_tensor("xt", [F_IN, N], f32, kind="ExternalInput").ap()
    w1e_d = nc.dram_tensor("w1e", [F_IN, C1], f32, kind="ExternalInput").ap()
    w2e_d = nc.dram_tensor("w2e", [HID, C2], f32, kind="ExternalInput").ap()
    # [M^T | adj^T] for this core's row-block, fp16: [N, 2*NLOC]
    amt_d = nc.dram_tensor("amt", [N, 2 * NLOC], f16, kind="ExternalInput").ap()
    # one-hot row-tile selector: sel[t, k] = 1 iff global tile t is own tile k
    sel_d = nc.dram_tensor("sel", [NT, TLOC], f32, kind="ExternalInput").ap()
    a_out_d = nc.dram_tensor("a_out", [NLOC, N], f32, kind="ExternalOutput").ap()
    z_out_d = nc.dram_tensor("z_out", [NLOC, EMB], f32, kind="ExternalOutput").ap()

    groups = [list(range(NCORES))]

    with tile.TileContext(nc) as tc:
        cst = tc.alloc_tile_pool(name="const", bufs=1)
        dramp = tc.alloc_tile_pool(name="dram", bufs=1, space="DRAM")

        ident = cst.tile([P, P], f32)
        masks.make_identity(nc, ident[:])
        ones_col = cst.tile([1, P], f32)
        nc.gpsimd.memset(ones_col[:], 1.0)
        sel_sb = cst.tile([NT, TLOC], f32)
        nc.sync.dma_start(out=sel_sb[:], in_=sel_d[:])
        w1e_sb = cst.tile([P, KH1 * C1], f32)
        for k in range(KH1):
            nc.sync.dma_start(
                out=w1e_sb[:, k * C1:(k + 1) * C1], in_=w1e_d[k * P:(k + 1) * P, :]
            )
        w2e_sb = cst.tile([P, KH2 * C2], f32)
        for k in range(KH2):
            nc.sync.dma_start(
                out=w2e_sb[:, k * C2:(k + 1) * C2], in_=w2e_d[k * P:(k + 1) * P, :]
            )

        # DRAM bounce buffers for collectives
        h2gin = dramp.tile([NLOC, C2], f32)
        h2g = dramp.tile([N, C2], f32, addr_space="Shared")
        ztin = dramp.tile([EMB, NLOC], f32)
        ztg = dramp.tile([NCORES * EMB, NLOC], f32, addr_space="Shared")

        def build_fs_bcast(fs_all, psumpool, smallpool, dst_dt, name):
            """fs_all [P, NT] f32 (col t = fs of global tile t's 128 rows)
            -> FS bcast [P, NLOC] of own rows, dst_dt."""
            fsT_p = psumpool.tile([NT, P], f32, name=f"{name}_fsT_p")
            nc.tensor.transpose(fsT_p[:], fs_all[:], ident[:])
            fsT_sb = smallpool.tile([NT, P], f32, name=f"{name}_fsT_sb")
            nc.vector.tensor_copy(fsT_sb[:], fsT_p[:])
            fso_p = psumpool.tile([TLOC, P], f32, name=f"{name}_fso_p")
            nc.tensor.matmul(fso_p[:], sel_sb[:], fsT_sb[:])
            fso_sb = smallpool.tile([TLOC, P], f32, name=f"{name}_fso_sb")
            nc.vector.tensor_copy(fso_sb[:], fso_p[:])
            # flatten own-row fs to one partition-0 row (matmul operands need
            # base_partition 0), then rank-1 broadcast across partitions
            fso_row = smallpool.tile([1, NLOC], f32, name=f"{name}_fso_row")
            for s in range(TLOC):
                nc.sync.dma_start(
                    out=fso_row[0:1, s * P:(s + 1) * P], in_=fso_sb[s:s + 1, :]
                )
            fsb_p = psumpool.tile([P, NLOC], f32, name=f"{name}_fsb_p")
            for s0 in range(0, NLOC, 512):
                w = min(512, NLOC - s0)
                nc.tensor.matmul(
                    fsb_p[:, s0:s0 + w], ones_col[:], fso_row[0:1, s0:s0 + w]
                )
            fsb = smallpool.tile([P, NLOC], dst_dt, name=f"{name}_fsb")
            nc.scalar.copy(fsb[:], fsb_p[:])
            return fsb

        def elu_inplace(dst, src_ap, tmpool, w, name):
            """dst [P, w] f32 <- elu(src_ap) ; elu(x)=relu(x)+exp(min(x,0))-1"""
            tmin = tmpool.tile([P, w], f32, name=f"{name}_tmin")
            nc.vector.tensor_scalar_min(tmin[:], src_ap, 0.0)
            texp = tmpool.tile([P, w], f32, name=f"{name}_texp")
            nc.scalar.activation(texp[:], tmin[:], AF.Exp)
            tmax = tmpool.tile([P, w], f32, name=f"{name}_tmax")
            nc.vector.tensor_scalar_max(tmax[:], src_ap, 0.0)
            nc.vector.scalar_tensor_tensor(
                dst, texp[:], -1.0, tmax[:], op0=ALU.add, op1=ALU.add
            )

        def gat_layer(hb, fn_all, FS, RW, acc, accp):
            """One GAT layer's score+PV sweep over all NT j-tiles.

            hb:  [P, NT*RW] bf16  (rhs tiles: h | ones per j-tile)
            fn_all: [P, NT] f32   (fn scalar per j-tile)
            FS:  [P, NLOC] f16    (own-row fs broadcast)
            acc: list of TLOC psum tiles [P, RW]
            """
            ldp = tc.alloc_tile_pool(name="ld", bufs=3)
            wkp = tc.alloc_tile_pool(name="wk", bufs=3)
            for j in range(NT):
                amt_t = ldp.tile([P, 2 * NLOC], f16, tag="amt", name="amt_t")
                nc.sync.dma_start(out=amt_t[:], in_=amt_d[j * P:(j + 1) * P, :])
                m_ap = amt_t[:, 0:NLOC]
                adj_ap = amt_t[:, NLOC:2 * NLOC]
                t_t = wkp.tile([P, NLOC], f16, tag="t", name="t_t")
                nc.vector.scalar_tensor_tensor(
                    t_t[:], FS[:], fn_all[:, j:j + 1], m_ap,
                    op0=ALU.add, op1=ALU.mult,
                )
                l_t = wkp.tile([P, NLOC], f16, tag="l", name="l_t")
                if True:  # TODO: Lrelu-ACT on odd tiles gave wrong results; investigate
                    nc.vector.scalar_tensor_tensor(
                        l_t[:], t_t[:], ALPHA, t_t[:], op0=ALU.mult, op1=ALU.max
                    )
                else:
                    nc.scalar.activation(l_t[:], t_t[:], AF.Lrelu, alpha=ALPHA)
                p_t = wkp.tile([P, NLOC], bf16, tag="p", name="p_t")
                nc.scalar.activation(p_t[:], l_t[:], AF.Exp)
                pb_t = wkp.tile([P, NLOC], bf16, tag="pb", name="pb_t")
                nc.vector.scalar_tensor_tensor(
                    pb_t[:], p_t[:], 1.0, adj_ap, op0=ALU.mult, op1=ALU.mult
                )
                hb_j = hb[:, j * RW:(j + 1) * RW]
                for c in range(TLOC):
                    nc.tensor.matmul(
                        acc[c][:], pb_t[:, c * P:(c + 1) * P], hb_j,
                        start=(j == 0), stop=(j == NT - 1),
                    )
            wkp.release()
            ldp.release()

        # ================= Phase A: h1' = x @ W1e =================
        p1 = tc.alloc_tile_pool(name="l1persist", bufs=1)
        hb1 = p1.tile([P, NT * R1], bf16)
        fn1_all = p1.tile([P, NT], f32)
        fs1_all = p1.tile([P, NT], f32)

        pha_ld = tc.alloc_tile_pool(name="pha_ld", bufs=8)
        pha_ps = tc.alloc_tile_pool(name="pha_ps", bufs=2, space="PSUM")
        IG = 8 if NT % 8 == 0 else 1  # i-tiles per x load group
        for g in range(NT // IG):
            xts = []
            for k in range(KH1):
                xt_t = pha_ld.tile([P, IG * P], f32, tag="xt", name=f"xt_{k}")
                nc.sync.dma_start(
                    out=xt_t[:],
                    in_=xt_d[k * P:(k + 1) * P, g * IG * P:(g + 1) * IG * P],
                )
                xts.append(xt_t)
            for tl in range(IG):
                it = g * IG + tl
                h1p = pha_ps.tile([P, C1], f32, tag="h1p", name="h1p")
                for k in range(KH1):
                    nc.tensor.matmul(
                        h1p[:], xts[k][:, tl * P:(tl + 1) * P],
                        w1e_sb[:, k * C1:(k + 1) * C1],
                        start=(k == 0), stop=(k == KH1 - 1),
                    )
                nc.scalar.copy(hb1[:, it * R1:it * R1 + HID], h1p[:, 0:HID])
                nc.vector.tensor_copy(fs1_all[:, it:it + 1], h1p[:, HID:HID + 1])
                nc.vector.tensor_copy(fn1_all[:, it:it + 1], h1p[:, HID + 1:HID + 2])
        # ones column of each rhs tile
        hb1_3d = hb1[:].rearrange("p (t c) -> p t c", c=R1)
        nc.gpsimd.memset(hb1_3d[:, :, HID:HID + 1], 1.0)
        pha_ps.release()
        pha_ld.release()

        # ================= Phase B: FS1 broadcast =================
        phb_ps = tc.alloc_tile_pool(name="phb_ps", bufs=1, space="PSUM")
        FS1 = build_fs_bcast(fs1_all, phb_ps, p1, f16, "fs1")
        phb_ps.release()

        # ================= Phase C/D: layer 1 sweep + epilogue =================
        acc1p = tc.alloc_tile_pool(name="acc1", bufs=1, space="PSUM")
        acc1 = [acc1p.tile([P, R1], f32, name=f"acc1_{c}") for c in range(TLOC)]
        gat_layer(hb1, fn1_all, FS1, R1, acc1, acc1p)

        houts = p1.tile([P, TLOC * HID], f32)
        phd = tc.alloc_tile_pool(name="phd", bufs=2)
        for c in range(TLOC):
            rec = phd.tile([P, 1], f32, tag="rec", name="rec")
            nc.vector.reciprocal(rec[:], acc1[c][:, HID:HID + 1])
            hsc = phd.tile([P, HID], f32, tag="hsc", name="hsc")
            nc.vector.tensor_scalar(
                hsc[:], acc1[c][:, 0:HID], rec[:], None, op0=ALU.mult
            )
            elu_inplace(
                houts[:, c * HID:(c + 1) * HID], hsc[:], phd, HID, f"elu1_{c}"
            )
        phd.release()
        acc1p.release()

        # ====== Phase E: h1o^T, h2' = h1o @ W2e, AllGather ======
        phe_ps = tc.alloc_tile_pool(name="phe_ps", bufs=2, space="PSUM")
        phe = tc.alloc_tile_pool(name="phe", bufs=2)
        h1T = p1.tile([P, KH2 * NLOC], f32)  # d-chunk k at cols [k*NLOC, (k+1)*NLOC)
        for c in range(TLOC):
            for k in range(KH2):
                tp = phe_ps.tile([P, P], f32, tag="tp", name="tp")
                nc.tensor.transpose(
                    tp[:], houts[:, c * HID + k * P:c * HID + (k + 1) * P], ident[:]
                )
                nc.vector.tensor_copy(
                    h1T[:, k * NLOC + c * P:k * NLOC + (c + 1) * P], tp[:]
                )
        for c in range(TLOC):
            h2p = phe_ps.tile([P, C2], f32, tag="h2p", name="h2p")
            for k in range(KH2):
                nc.tensor.matmul(
                    h2p[:], h1T[:, k * NLOC + c * P:k * NLOC + (c + 1) * P],
                    w2e_sb[:, k * C2:(k + 1) * C2],
                    start=(k == 0), stop=(k == KH2 - 1),
                )
            h2sb = phe.tile([P, C2], f32, tag="h2sb", name="h2sb")
            nc.vector.tensor_copy(h2sb[:], h2p[:])
            nc.sync.dma_start(out=h2gin[c * P:(c + 1) * P, :], in_=h2sb[:])
        nc.gpsimd.collective_compute(
            "AllGather", ALU.bypass, replica_groups=groups,
            ins=[h2gin.opt()], outs=[h2g.opt()],
        )
        phe.release()
        phe_ps.release()
        p1.release()

        # ================= Phase F: layer-2 setup =================
        p2 = tc.alloc_tile_pool(name="l2persist", bufs=1)
        hb2 = p2.tile([P, NT * R2], bf16)
        fn2_all = p2.tile([P, NT], f32)
        fs2_all = p2.tile([P, NT], f32)
        h2sb_all = p2.tile([P, NT * C2], f32)
        nc.sync.dma_start(
            out=h2sb_all[:].rearrange("p (t c) -> p t c", c=C2),
            in_=h2g[:].rearrange("(t p) c -> p t c", p=P),
        )
        h2_3d = h2sb_all[:].rearrange("p (t c) -> p t c", c=C2)
        hb2_3d = hb2[:].rearrange("p (t c) -> p t c", c=R2)
        nc.scalar.copy(hb2_3d[:, :, 0:EMB], h2_3d[:, :, 0:EMB])
        nc.gpsimd.memset(hb2_3d[:, :, EMB:EMB + 1], 1.0)
        nc.vector.tensor_copy(fs2_all[:], h2_3d[:, :, EMB])
        nc.vector.tensor_copy(fn2_all[:], h2_3d[:, :, EMB + 1])

        phf_ps = tc.alloc_tile_pool(name="phf_ps", bufs=1, space="PSUM")
        FS2 = build_fs_bcast(fs2_all, phf_ps, p2, f16, "fs2")
        phf_ps.release()

        # ================= Phase G/H: layer 2 sweep + z =================
        acc2p = tc.alloc_tile_pool(name="acc2", bufs=1, space="PSUM")
        acc2 = [acc2p.tile([P, R2], f32, name=f"acc2_{c}") for c in range(TLOC)]
        gat_layer(hb2, fn2_all, FS2, R2, acc2, acc2p)

        zT_own = p2.tile([EMB, NLOC], f32)
        zloc = p2.tile([P, TLOC * EMB], f32)
        phh = tc.alloc_tile_pool(name="phh", bufs=2)
        for c in range(TLOC):
            rec = phh.tile([P, 1], f32, tag="rec2", name="rec2")
            nc.vector.reciprocal(rec[:], acc2[c][:, EMB:EMB + 1])
            hsc = phh.tile([P, EMB], f32, tag="hsc2", name="hsc2")
            nc.vector.tensor_scalar(
                hsc[:], acc2[c][:, 0:EMB], rec[:], None, op0=ALU.mult
            )
            hat = phh.tile([P, EMB], f32, tag="hat", name="hat")
            elu_inplace(hat[:], hsc[:], phh, EMB, f"elu2_{c}")
            # l2 normalize: z = hat * rsqrt(sum(hat^2))
            dummy = phh.tile([P, EMB], f32, tag="dummy", name="dummy")
            ss = phh.tile([P, 1], f32, tag="ss", name="ss")
            nc.vector.scalar_tensor_tensor(
                dummy[:], hat[:], 1.0, hat[:], op0=ALU.mult, op1=ALU.mult,
                accum_out=ss[:],
            )
            w0 = phh.tile([P, 1], f32, tag="w0", name="w0")
            nc.vector.reciprocal(w0[:], ss[:])
            v0 = phh.tile([P, 1], f32, tag="v0", name="v0")
            nc.scalar.activation(v0[:], w0[:], AF.Sqrt)
            # one Newton step for 1/sqrt(ss): v1 = v0*(1.5 - 0.5*ss*v0^2)
            aa = phh.tile([P, 1], f32, tag="aa", name="aa")
            nc.vector.scalar_tensor_tensor(
                aa[:], v0[:], 1.0, v0[:], op0=ALU.mult, op1=ALU.mult
            )
            bb = phh.tile([P, 1], f32, tag="bb", name="bb")
            nc.vector.tensor_scalar(aa[:], aa[:], ss[:], None, op0=ALU.mult)
            nc.vector.tensor_scalar(bb[:], aa[:], -0.5, 1.5, op0=ALU.mult, op1=ALU.add)
            v1 = phh.tile([P, 1], f32, tag="v1", name="v1")
            nc.vector.tensor_scalar(v1[:], v0[:], bb[:], None, op0=ALU.mult)
            zt = zloc[:, c * EMB:(c + 1) * EMB]
            nc.vector.tensor_scalar(zt, hat[:], v1[:], None, op0=ALU.mult)
            nc.sync.dma_start(out=z_out_d[c * P:(c + 1) * P, :], in_=zt)
        phh.release()
        acc2p.release()
        phh_ps = tc.alloc_tile_pool(name="phh_ps", bufs=2, space="PSUM")
        for c in range(TLOC):
            ztp = phh_ps.tile([EMB, P], f32, tag="ztp", name="ztp")
            nc.tensor.transpose(ztp[:], zloc[:, c * EMB:(c + 1) * EMB], ident[:])
            nc.vector.tensor_copy(zT_own[:, c * P:(c + 1) * P], ztp[:])
        phh_ps.release()

        nc.sync.dma_start(out=ztin[:], in_=zT_own[:])
        nc.gpsimd.collective_compute(
            "AllGather", ALU.bypass, replica_groups=groups,
            ins=[ztin.opt()], outs=[ztg.opt()],
        )

        # ================= Phase I: decode A = sigmoid(z z^T) =================
        zts = p2.tile([EMB, N], f32)
        for b in range(NCORES):
            nc.sync.dma_start(
                out=zts[:, b * NLOC:(b + 1) * NLOC],
                in_=ztg[b * EMB:(b + 1) * EMB, :],
            )
        DEC_G = min(2048, N)   # output cols per psum group
        NQ = DEC_G // 512
        dec_ps = tc.alloc_tile_pool(name="dec_ps", bufs=2, space="PSUM")
        dec_sb = tc.alloc_tile_pool(name="dec_sb", bufs=3)
        for c in range(TLOC):
            for g in range(N // DEC_G):
                ps = dec_ps.tile([P, DEC_G], f32, tag="ps", name="ps")
                for q in range(NQ):
                    j0 = g * DEC_G + q * 512
                    nc.tensor.matmul(
                        ps[:, q * 512:(q + 1) * 512],
                        zT_own[:, c * P:(c + 1) * P], zts[:, j0:j0 + 512],
                    )
                asb = dec_sb.tile([P, DEC_G], f32, tag="asb", name="asb")
                nc.scalar.activation(asb[:], ps[:], AF.Sigmoid)
                nc.sync.dma_start(
                    out=a_out_d[c * P:(c + 1) * P, g * DEC_G:(g + 1) * DEC_G],
                    in_=asb[:],
                )
        dec_sb.release()
        dec_ps.release()
        p2.release()
        dramp.release()
        cst.release()

    nc.compile()
    return nc


def _prep_inputs(x, adj, M, W1, a_self1, a_neighs1, W2, a_self2, a_neighs2):
    N = x.shape[0]
    NLOC = N // NCORES
    NT = N // P
    TLOC = NLOC // P
    x = np.asarray(x, np.float32)
    w1e = np.concatenate(
        [W1, W1 @ a_self1, W1 @ a_neighs1], axis=1
    ).astype(np.float32)
    w2e = np.concatenate(
        [W2, W2 @ a_self2, W2 @ a_neighs2], axis=1
    ).astype(np.float32)
    xt = np.ascontiguousarray(x.T)
    in_maps = []
    for c in range(NCORES):
        rows = slice(c * NLOC, (c + 1) * NLOC)
        amt = np.empty((N, 2 * NLOC), np.float16)
        amt[:, :NLOC] = np.asarray(M, np.float32)[rows, :].T
        amt[:, NLOC:] = np.asarray(adj, np.float32)[rows, :].T
        sel = np.zeros((NT, TLOC), np.float32)
        for k in range(TLOC):
            sel[c * TLOC + k, k] = 1.0
        in_maps.append(
            {"xt": xt, "w1e": w1e, "w2e": w2e, "amt": amt, "sel": sel}
        )
    return in_maps


def _pjrt_runner(nc, in_maps):
    """Mirror of bass2jax.run_bass_via_pjrt's multi-core path, but returns a
    reusable jitted callable with device-resident operands so repeated
    executions can be timed without re-tracing or re-uploading inputs."""
    import jax
    from concourse import bass2jax as b2j
    from concourse import mybir
    from jax.experimental.shard_map import shard_map
    from jax.sharding import Mesh, PartitionSpec

    b2j.install_neuronx_cc_hook()
    n_cores = len(in_maps)
    partition_name = (
        nc.partition_id_tensor.name if nc.partition_id_tensor else None
    )

    in_names, out_names, out_avals, zero_outs = [], [], [], []
    for alloc in nc.m.functions[0].allocations:
        if not isinstance(alloc, mybir.MemoryLocationSet):
            continue
        name = alloc.memorylocations[0].name
        if alloc.kind == "ExternalInput":
            if name != partition_name:
                in_names.append(name)
        elif alloc.kind == "ExternalOutput":
            out_names.append(name)
            shape = tuple(alloc.tensor_shape)
            dtype = mybir.dt.np(alloc.dtype)
            out_avals.append(jax.core.ShapedArray(shape, dtype))
            zero_outs.append(np.zeros(shape, dtype))
    n_params = len(in_names)
    all_names = in_names + out_names
    if partition_name is not None:
        all_names = all_names + [partition_name]

    def _body(*args):
        operands = list(args)
        if partition_name is not None:
            operands.append(b2j.partition_id_tensor())
        return tuple(
            b2j._bass_exec_p.bind(
                *operands,
                out_avals=tuple(out_avals),
                in_names=tuple(all_names),
                out_names=tuple(out_names),
                lowering_input_output_aliases=(),
                sim_require_finite=True,
                sim_require_nnan=True,
                nc=nc,
            )
        )

    devices = jax.devices()[:n_cores]
    mesh = Mesh(np.asarray(devices), ("core",))
    nin = n_params + len(out_names)
    sharded = jax.jit(
        shard_map(
            _body, mesh=mesh,
            in_specs=(PartitionSpec("core"),) * nin,
            out_specs=(PartitionSpec("core"),) * len(out_names),
            check_rep=False,
        ),
        keep_unused=True,
    )
    sh = jax.sharding.NamedSharding(mesh, PartitionSpec("core"))
    dev_in = [
        jax.device_put(
            np.concatenate([np.asarray(m[name]) for m in in_maps], axis=0), sh
        )
        for name in in_names
    ] + [
        jax.device_put(
            np.zeros((n_cores * z.shape[0], *z.shape[1:]), z.dtype), sh
        )
        for z in zero_outs
    ]

    def run_once():
        outs = sharded(*dev_in)
        jax.block_until_ready(outs)
        return outs

    def decode(outs):
        return [
            {
                name: np.asarray(outs[i]).reshape(n_cores, *out_avals[i].shape)[c]
                for i, name in enumerate(out_names)
            }
            for c in range(n_cores)
        ]

    return run_once, decode


def run(inputs, bench=0):
    """Returns ((A_pred, z), exec_time_ns or None)."""
    import time

    x = np.asarray(inputs["x"])
    N, F_IN = x.shape
    HID = inputs["W1"].shape[1]
    EMB = inputs["W2"].shape[1]
    key = (N, F_IN, HID, EMB)
    if key not in _compiled_cache:
        _compiled_cache[key] = _build_program(N, F_IN, HID, EMB)
    nc = _compiled_cache[key]
    in_maps = _prep_inputs(**{k: np.asarray(v) for k, v in inputs.items()})
    run_once, decode = _pjrt_runner(nc, in_maps)
    outs = run_once()
    exec_ns = None
    if bench:
        ts = []
        for _ in range(bench):
            t0 = time.perf_counter()
            outs = run_once()
            ts.append(time.perf_counter() - t0)
        exec_ns = int(min(ts) * 1e9)
    results = decode(outs)
    A = np.concatenate([results[c]["a_out"] for c in range(NCORES)], axis=0)
    z = np.concatenate([results[c]["z_out"] for c in range(NCORES)], axis=0)
    return (A, z), exec_ns


def kernel(**inputs):
    out, _ = run(inputs, trace=False)
    return out
